# revision 27
# baseline (speedup 1.0000x reference)
"""Trainium2 Bass kernel for nn_Decoder_481036337511.

Computation: dic = normalized real dictionary [T=1024, 1+4*4096] built from
rr/theta; out = einsum('tk,bkd->btd', dic, x) with x [4, 16385, 2048].

Strategy (8 cores, tensor parallel on D):
  - Dictionary structure [ones, A, S*A, B, S*B] with A = r^t cos(t th),
    B = r^t sin(t th), S = diag((-1)^t); the column norms of S*A equal
    those of A.  With u=x1+x2, v=x1-x2, w=x3+x4, z=x3-x4:
       out[even t] = Abar_e @ u + Bbar_e @ w + x0/32
       out[odd  t] = Abar_o @ v + Bbar_o @ z + x0/32
    which halves the GEMM FLOPs.
  - Sparsity: normalized columns decay like r^t.  With poles sorted by
    descending r, each 128-pole chunk only needs a short prefix of the
    512 per-parity time rows (drop budget 5e-4 relative).
  - The whole x side ships as fp8-e3m4 (4 mantissa bits): deterministic
    ~1.3e-2 rel error vs the 2e-2 gate, and only ~17 MB/core of HBM
    traffic.  The dictionary is fp16 (exact fp32 build on host); the odd
    parity dictionary is derived on the vector engine via angle addition.
  - The kernel is DMA-bound (~20 MB vs ~310-360 GB/s per core): x streams
    on the sync HWDGE queue; dictionary and output DMAs ride the scalar
    HWDGE queue so the x stream never stalls.
  - The ones-column bias (x0/32) and the parity/t-tile interleave back
    to [B, T, D] are applied on the host.
"""

import numpy as np
from contextlib import ExitStack

import concourse.bass as bass
import concourse.bacc as bacc
import concourse.mybir as mybir
from concourse import tile
from concourse import bass_utils

F32 = mybir.dt.float32
F16 = mybir.dt.float16
F8E3 = mybir.dt.float8e3
AF = mybir.ActivationFunctionType
OP = mybir.AluOpType

N_CORES = 8
T = 1024
NP_ = 4096          # poles
B = 4
D = 2048
DSH = D // N_CORES  # 256 d columns per core
G_ = 16             # chunk pair groups (32 chunks of 128 poles)
NC_ = 32            # pole chunks
TT = 4              # t tiles of 128 per parity (512 t per parity)
BD = B * DSH        # 1024 (b,d) columns per core
DROP_BUDGET = (5e-4) ** 2 * 16385   # allowed dropped Frobenius mass
E3_CLIP = 14.0      # keep |x| under e3m4 max normal (15.5)


def _sched_layout(L):
    """Per-chunk kept t-rows (L[c]*32) -> tile widths and column offsets.

    Returns (tiles, cw): tiles[c] = list of (tt, width, off_a, off_b)
    and cw = total flat dictionary columns per parity.
    """
    tiles = []
    cw = 0
    for c in range(NC_):
        rows = 32 * L[c]
        tl = []
        tt = 0
        while rows > 0:
            w = min(128, rows)
            tl.append((tt, w, cw, cw + w))
            cw += 2 * w
            rows -= w
            tt += 1
        tiles.append(tl)
    return tiles, cw


def _quad_starts(tiles):
    """4-chunk supergroups made of single-tile 32-row chunks; these run
    column-tiled on the PE array (one chunk per 32-column strip)."""
    def small(c):
        return len(tiles[c]) == 1 and tiles[c][0][1] == 32

    return [c for c in range(4, NC_ - 3, 4)
            if all(small(c + k) for k in range(4))]


def build_kernel_nc(L):
    tiles, CW = _sched_layout(L)
    qstarts = _quad_starts(tiles)
    quadset = {qc + k for qc in qstarts for k in range(4)}
    first_c = {}
    last_c = {}
    for c in range(NC_):
        if c in quadset:
            continue
        for (tt, w, oa, ob_) in tiles[c]:
            first_c.setdefault(tt, c)
            last_c[tt] = c

    nc = bacc.Bacc("TRN2", target_bir_lowering=False, debug=False)

    ex_d = nc.dram_tensor("ex", [G_, 128, 2, 2, B, DSH], F8E3,
                          kind="ExternalInput")
    ox_d = nc.dram_tensor("ox", [G_, 128, 2, 2, B, DSH], F8E3,
                          kind="ExternalInput")
    dc_d = nc.dram_tensor("dc", [128, CW], F16, kind="ExternalInput")
    sc_d = nc.dram_tensor("sc", [128, 3, NC_], F32, kind="ExternalInput")
    # 0/1 matrix folding the 4 column-group partials back onto t-rows
    # 0:32: mg[p, r] = (p % 32 == r).
    mg_d = nc.dram_tensor("mg", [128, 32], F16, kind="ExternalInput")
    out_d = nc.dram_tensor("out", [2, 128, TT, BD], F16,
                           kind="ExternalOutput")

    with tile.TileContext(nc) as tc, ExitStack() as ctx:
        xp = ctx.enter_context(tc.tile_pool(name="xp", bufs=16))
        dp = ctx.enter_context(tc.tile_pool(name="dp", bufs=1))
        op_ = ctx.enter_context(tc.tile_pool(name="op", bufs=8))
        wp = ctx.enter_context(tc.tile_pool(name="wp", bufs=1))
        psp = ctx.enter_context(
            tc.tile_pool(name="ps", bufs=1, space=bass.MemorySpace.PSUM))

        # PE warm-up: HAM un-throttles (1.2 -> 2.4 GHz) only after ~3.4us
        # of sustained PE activity; burn the initial DMA window with
        # dummy matmuls (1-col stationary -> ~free LDWEIGHTS).
        wt = wp.tile([128, 257], F16, tag="wt", name="wt")
        nc.vector.memset(wt[:], 0.0)
        wps = psp.tile([128, 512], F32, tag="ps00", name="wps")
        for i in range(26):
            nc.tensor.matmul(wps[0:1, 0:256], wt[:, 0:1], wt[:, 1:257],
                             start=True, stop=True)

        # Dictionary loads ride the scalar queue so the sync queue streams
        # x back-to-back; the first chunks' tiles load first so the first
        # matmuls aren't gated on the full dictionary.
        nsplit = min(tiles[1][-1][3] if NC_ > 1 else CW, CW)

        dt0 = dp.tile([128, CW], F16, tag="d0", name="dt0")
        nc.scalar.dma_start(dt0[:, 0:nsplit], dc_d[:, 0:nsplit])
        if nsplit < CW:
            nc.scalar.dma_start(dt0[:, nsplit:], dc_d[:, nsplit:])
        sc_t = wp.tile([128, 3, NC_], F32, tag="sc", name="sc_t")
        nc.scalar.dma_start(sc_t[:], sc_d[:])
        mg_t = wp.tile([128, 32], F16, tag="mg", name="mg_t")
        nc.scalar.dma_start(mg_t[:], mg_d[:])

        # Derive the odd-parity dictionary on the (idle) vector engine via
        # angle addition -- saves 0.85 MB of HBM traffic:
        #   A_o = s1*A_e - s2*B_e,  B_o = s1*B_e + s3*A_e
        # with per-pole s1 = r cos(th), s2 = r sin(th) G3/G1,
        # s3 = r sin(th) G1/G3.
        dt1 = dp.tile([128, CW], F16, tag="d1", name="dt1")
        tp_ = ctx.enter_context(tc.tile_pool(name="tp", bufs=2))
        for c in range(NC_):
            for (tt, w, oa, ob_) in tiles[c]:
                tmp = tp_.tile([128, 128], F16, tag="tmp", name="tmp")
                nc.vector.tensor_scalar(tmp[:, 0:w], dt0[:, ob_:ob_ + w],
                                        sc_t[:, 1, c:c + 1], None,
                                        op0=OP.mult)
                nc.vector.scalar_tensor_tensor(
                    dt1[:, oa:oa + w], dt0[:, oa:oa + w],
                    sc_t[:, 0, c:c + 1], tmp[:, 0:w],
                    op0=OP.mult, op1=OP.subtract)
                tmp2 = tp_.tile([128, 128], F16, tag="tmp", name="tmp2")
                nc.vector.tensor_scalar(tmp2[:, 0:w], dt0[:, oa:oa + w],
                                        sc_t[:, 2, c:c + 1], None,
                                        op0=OP.mult)
                nc.vector.scalar_tensor_tensor(
                    dt1[:, ob_:ob_ + w], dt0[:, ob_:ob_ + w],
                    sc_t[:, 0, c:c + 1], tmp2[:, 0:w],
                    op0=OP.mult, op1=OP.add)
        dts = [dt0, dt1]

        for par, xd in enumerate((ex_d, ox_d)):
            # PSUM: per h, bank 0 holds t-rows 0:128 (tt0) and a 3-bank
            # tile holds tt1-3 contiguously so they evacuate in one op.
            # The tt1 bank doubles as the column-tiled partial bank for
            # the 32-row chunks once tt1-3 retire (first group), and the
            # tt3/h1 bank hosts tiny keep-warm matmuls.
            ps = [[None, None] for _ in range(TT)]
            tails = []
            for h in range(2):
                ps[0][h] = psp.tile([128, 512], F32, tag=f"ps0{h}",
                                    name=f"ps0{h}")
                tl3 = psp.tile([128, 3, 512], F32, tag=f"pst{h}",
                               name=f"pst{par}{h}")
                tails.append(tl3)
                for tt in range(1, TT):
                    ps[tt][h] = tl3[:, tt - 1]
            aux = [tails[h][:, 0] for h in range(2)]
            warm_ps = tails[1][:, 2]
            nquad = len(qstarts)
            tail_last = max([last_c.get(tt, 0) for tt in range(1, TT)],
                            default=0)
            dt = dts[par]
            if par == 1:
                # Keep HAM warm across the parity transition.
                for i in range(8):
                    nc.tensor.matmul(wps[0:1, 0:256], wt[:, 0:1],
                                     wt[:, 1:257], start=True, stop=True)
            for g in range(G_):
                xt = xp.tile([128, 2, 2, B, DSH], F8E3, tag="x", name="xt")
                last_g = par == 1 and g == G_ - 1
                if (par == 0 and g == 0) or last_g:
                    # Halve the first/last transfer so the adjacent
                    # matmuls wait on 0.25 MB, not 0.5 MB.
                    nc.sync.dma_start(xt[:, 0], xd[g, :, 0])
                    nc.sync.dma_start(xt[:, 1], xd[g, :, 1])
                else:
                    nc.sync.dma_start(xt[:], xd[g])
                c0g = 2 * g
                if c0g in qstarts or c0g - 2 in qstarts:
                    qc = c0g if c0g in qstarts else c0g - 2
                    qi = qstarts.index(qc)
                    if c0g in qstarts:
                        # first half of the quad: stash the tile, issue
                        # a keep-warm matmul while the second half lands.
                        # The moving operand reads the fresh x tile so
                        # the scheduler pins it into the DMA-wait gap.
                        pend = xt
                        if g >= 2:
                            nc.tensor.matmul(warm_ps[0:1, :],
                                             wt[:, 0:1], xt[:, 0, 0, 0:2],
                                             start=True, stop=True)
                        continue
                    xts = [pend, xt]
                    # 4 waves of 4 matmuls; each wave hits all 4 column
                    # groups so the strips run concurrently.  The last
                    # quad runs h-major so aux[0] retires two waves
                    # early and its evacuation overlaps aux[1]'s tail.
                    if qi == nquad - 1:
                        waves = [(ab, h) for h in range(2)
                                 for ab in range(2)]
                    else:
                        waves = [(ab, h) for ab in range(2)
                                 for h in range(2)]
                    for ab, h in waves:
                        for k in range(4):
                            c = qc + k
                            tt, w, oa, ob2 = tiles[c][0]
                            off = oa if ab == 0 else ob2
                            nc.tensor.matmul(
                                aux[h][32 * k:32 * k + w, :],
                                dt[:, off:off + w],
                                xts[k // 2][:, k % 2, ab,
                                            2 * h:2 * h + 2],
                                # first write of each column strip must
                                # clear its stale has_written bits (the
                                # clear is per-element, not per-bank).
                                start=(qi == 0 and ab == 0),
                                stop=(qi == nquad - 1 and h == 1
                                      and ab == 1 and k == 3),
                                tile_position=(0, 32 * k),
                            )
                    continue
                for j in range(2):
                    c = c0g + j
                    tl = tiles[c]
                    if par == 1 and c == 0 and len(tl) > 1:
                        # tt0 last: its bank frees only after the prior
                        # parity's merge + evacuation, and the PE queue
                        # is in-order.
                        tl = tl[1:] + tl[:1]
                    for (tt, w, oa, ob2) in tl:
                        for ab, off in ((0, oa), (1, ob2)):
                            for h in range(2):
                                nc.tensor.matmul(
                                    ps[tt][h][0:w, :],
                                    dt[:, off:off + w],
                                    xt[:, j, ab, 2 * h:2 * h + 2],
                                    start=(c == first_c[tt] and ab == 0),
                                    stop=(c == last_c[tt] and ab == 1
                                          and not (tt == 0 and nquad)),
                                )
                    if c == tail_last and TT > 1:
                        # tt1-3 retire here (only the big chunks reach
                        # them): one 3-bank copy + one DMA per h, on the
                        # scalar engine -- the vector engine is deep in
                        # the odd-dictionary derivation at this point.
                        for h in range(2):
                            ob3 = op_.tile([128, 3, 512], F16,
                                           tag="ob3", name="ob3")
                            nc.scalar.activation(ob3[:], tails[h][:],
                                                 AF.Identity, bias=0.0,
                                                 scale=1.0)
                            nc.scalar.dma_start(
                                out_d[par, :, 1:TT,
                                      h * 512:(h + 1) * 512], ob3[:])
            # Fold the 4 column-group partials onto t-rows 0:32 of the
            # main bank with a tiny 0/1 matmul, then evacuate tt0.  In
            # the final tail, spread the DMAs over the scalar AND the
            # (now idle) sync queue so the issues don't serialize.
            for h in range(2):
                if nquad:
                    axs = op_.tile([128, 512], F16, tag="ax", name="ax")
                    nc.vector.tensor_copy(axs[:], aux[h][:, :])
                    nc.tensor.matmul(ps[0][h][0:32, :], mg_t[:, 0:32],
                                     axs[:], start=False, stop=True)
                nq = 2 if par == 1 else 1
                ob = op_.tile([128, 512], F16, tag="ob", name="ob")
                for q in range(nq):
                    cs = slice(q * 512 // nq, (q + 1) * 512 // nq)
                    osl = out_d[par, :, 0, h * 512 + cs.start:
                                h * 512 + cs.stop]
                    if h == 0:
                        nc.scalar.activation(ob[:, cs], ps[0][h][:, cs],
                                             AF.Identity, bias=0.0,
                                             scale=1.0)
                        nc.scalar.dma_start(osl, ob[:, cs])
                    else:
                        nc.vector.tensor_copy(ob[:, cs], ps[0][h][:, cs])
                        eng = nc.sync if par == 1 else nc.scalar
                        eng.dma_start(osl, ob[:, cs])
    nc.compile()
    return nc


_NC_CACHE = {}


def _get_nc(L):
    key = tuple(L)
    if key not in _NC_CACHE:
        _NC_CACHE[key] = build_kernel_nc(L)
    return _NC_CACHE[key]


def _build_dict_halves(rr, theta):
    """Normalized Abar/Bbar [T, NP_] fp32, exactly as the reference."""
    i = np.arange(T, dtype=np.float32)[:, None]
    pw = rr[None, :] ** i
    ang = (i * theta[None, :]).astype(np.float32)
    c = np.cos(ang).astype(np.float32)
    s = np.sin(ang).astype(np.float32)
    sign = np.where(i % 2 == 0, np.float32(1.0), np.float32(-1.0))
    ones = np.ones((T, 1), np.float32)
    w1 = pw * c
    w3 = pw * s
    dic = np.concatenate([ones, w1, sign * w1, w3, sign * w3],
                         axis=1).astype(np.float32)
    G = np.linalg.norm(dic, axis=0)
    G = np.where(G == 0, np.sqrt(np.float32(T)), G).astype(np.float32)
    abar = dic[:, 1:1 + NP_] / G[None, 1:1 + NP_]
    bbar = dic[:, 1 + 2 * NP_:1 + 3 * NP_] / G[None, 1 + 2 * NP_:1 + 3 * NP_]
    return abar, bbar


def _schedule(abar, bbar):
    """Per-chunk kept prefix length (units of 32 per-parity t rows).

    Mass of (chunk, 32-row block) = sum over both parities of squared
    normalized entries (A and B blocks; the S*A / S*B blocks mirror
    them, scaling total and dropped mass alike).
    """
    sq = abar * abar + bbar * bbar                      # [T, NP_]
    m = sq.reshape(16, 64, NC_, 128).sum(axis=(1, 3))   # [block, chunk]
    tail = m[::-1].cumsum(axis=0)[::-1]                 # tail mass from block
    share = DROP_BUDGET / 2 / NC_
    L = []
    for c in range(NC_):
        keep = 16
        for l in range(1, 17):
            if l == 16 or tail[l, c] <= share:
                keep = l
                break
        L.append(keep)
    return L


def _pack_dict(abar, bbar, L):
    """-> [2par, 128p, CW] fp16 flat dictionary per the schedule."""
    tiles, CW = _sched_layout(L)
    at = np.ascontiguousarray(abar.T)   # [NP_ k, T]
    bt = np.ascontiguousarray(bbar.T)
    dc = np.empty((2, 128, CW), np.float16)
    for par in range(2):
        atp = at[:, par::2]             # [NP_, 512]
        btp = bt[:, par::2]
        for c in range(NC_):
            ks = slice(c * 128, (c + 1) * 128)
            for (tt, w, oa, ob_) in tiles[c]:
                ms = slice(tt * 128, tt * 128 + w)
                dc[par, :, oa:oa + w] = atp[ks, ms]
                dc[par, :, ob_:ob_ + w] = btp[ks, ms]
    return dc


def _pack_xside(a, b):
    """a,b [B, NP_, D] fp32 (sorted poles) -> per-core e3m4 parts
    [G_, 128p, 2j, 2uw, B, DSH]."""
    import ml_dtypes
    big = np.stack([a, b], axis=0)                     # [2uw, B, NP_, D]
    np.clip(big, -E3_CLIP, E3_CLIP, out=big)
    big = big.astype(ml_dtypes.float8_e3m4)
    r = big.reshape(2, B, G_, 2, 128, N_CORES, DSH)
    # (uw0, b1, g2, j3, p4, c5, d6) -> (c, g, p, j, uw, b, d)
    rt = np.ascontiguousarray(r.transpose(5, 2, 4, 3, 0, 1, 6))
    return [rt[c] for c in range(N_CORES)]


def kernel(rr, theta, x, trace=False, trace_kwargs=None):
    rr = np.ascontiguousarray(np.asarray(rr, dtype=np.float32))
    theta = np.ascontiguousarray(np.asarray(theta, dtype=np.float32))
    x = np.asarray(x, dtype=np.float32)

    order = np.argsort(-rr, kind="stable")
    abar, bbar = _build_dict_halves(rr, theta)
    abar = abar[:, order]
    bbar = bbar[:, order]
    L = _schedule(abar, bbar)
    dc = np.ascontiguousarray(_pack_dict(abar, bbar, L)[0])
    i = np.arange(T, dtype=np.float32)[:, None]
    pw = rr[None, :] ** i
    ang = (i * theta[None, :]).astype(np.float32)
    G1 = np.linalg.norm((pw * np.cos(ang)).astype(np.float32), axis=0)
    G3 = np.linalg.norm((pw * np.sin(ang)).astype(np.float32), axis=0)
    G1 = np.where(G1 == 0, np.float32(np.sqrt(T)), G1)
    G3 = np.where(G3 == 0, np.float32(np.sqrt(T)), G3)
    rs, ths = rr[order], theta[order]
    g1s, g3s = G1[order], G3[order]
    s1 = (rs * np.cos(ths)).astype(np.float32)
    sf = (rs * np.sin(ths)).astype(np.float32)
    sc = np.stack([s1, sf * g3s / g1s, sf * g1s / g3s], axis=0)
    sc = np.ascontiguousarray(
        sc.reshape(3, NC_, 128).transpose(2, 0, 1).astype(np.float32))

    x1 = x[:, 1:1 + NP_][:, order]
    x2 = x[:, 1 + NP_:1 + 2 * NP_][:, order]
    x3 = x[:, 1 + 2 * NP_:1 + 3 * NP_][:, order]
    x4 = x[:, 1 + 3 * NP_:1 + 4 * NP_][:, order]
    u, w = x1 + x2, x3 + x4
    v, z = x1 - x2, x3 - x4
    # e3m4 max normal is 15.5; iid gaussian absmax is ~8, but guard by
    # rescaling everything if an adversarial input would clip hard.
    amax = max(np.abs(a).max() for a in (u, w, v, z))
    s_glob = np.float32(1.0)
    if amax > E3_CLIP:
        s_glob = np.float32(amax / E3_CLIP)
        inv = np.float32(1.0 / s_glob)
        u = u * inv
        w = w * inv
        v = v * inv
        z = z * inv
    ex_cores = _pack_xside(u, w)
    ox_cores = _pack_xside(v, z)

    mg = np.zeros((128, 32), np.float16)
    mg[np.arange(128), np.arange(128) % 32] = np.float16(1.0)

    nc = _get_nc(L)
    in_maps = []
    for c in range(N_CORES):
        im = {"ex": ex_cores[c], "ox": ox_cores[c], "dc": dc, "sc": sc,
              "mg": mg}
        in_maps.append(im)
    kw = {}
    if trace:
        kw = {"trace": True, "trace_kwargs": trace_kwargs or {}}
    try:
        res = bass_utils.run_bass_kernel_spmd(
            nc, in_maps, core_ids=list(range(N_CORES)), **kw)
    except Exception:
        # Transient device wedge (e.g. NRT_EXEC_UNIT_UNRECOVERABLE) --
        # one retry usually clears it.
        res = bass_utils.run_bass_kernel_spmd(
            nc, in_maps, core_ids=list(range(N_CORES)), **kw)

    out = np.empty((B, T, D), dtype=np.float32)
    for c in range(N_CORES):
        oc = res.results[c]["out"]           # [2, 128, TT, BD]
        dsl = slice(c * DSH, (c + 1) * DSH)
        for par in range(2):
            for tt in range(TT):
                blk = oc[par, :, tt].reshape(128, B, DSH)
                blk = blk.transpose(1, 0, 2)
                out[:, 256 * tt + par:256 * (tt + 1):2, dsl] = blk
    if s_glob != 1.0:
        out *= s_glob
    out += x[:, 0:1, :] * np.float32(1.0 / 32.0)
    if trace:
        return out, res
    return out


# revision 29
# speedup vs baseline: 1.0626x; 1.0626x over previous
"""Trainium2 Bass kernel for nn_Decoder_481036337511.

Computation: dic = normalized real dictionary [T=1024, 1+4*4096] built from
rr/theta; out = einsum('tk,bkd->btd', dic, x) with x [4, 16385, 2048].

Strategy (8 cores, tensor parallel on D):
  - Dictionary structure [ones, A, S*A, B, S*B] with A = r^t cos(t th),
    B = r^t sin(t th), S = diag((-1)^t); the column norms of S*A equal
    those of A.  With u=x1+x2, v=x1-x2, w=x3+x4, z=x3-x4:
       out[even t] = Abar_e @ u + Bbar_e @ w + x0/32
       out[odd  t] = Abar_o @ v + Bbar_o @ z + x0/32
    which halves the GEMM FLOPs.
  - Sparsity: normalized columns decay like r^t.  With poles sorted by
    descending r, each 128-pole chunk only needs a short prefix of the
    512 per-parity time rows (drop budget 5e-4 relative).
  - The whole x side ships as fp8-e3m4 (4 mantissa bits): deterministic
    ~1.3e-2 rel error vs the 2e-2 gate, and only ~17 MB/core of HBM
    traffic.  The dictionary is fp16 (exact fp32 build on host); the odd
    parity dictionary is derived on the vector engine via angle addition.
  - The kernel is DMA-bound (~20 MB vs ~310-360 GB/s per core): x streams
    on the sync HWDGE queue; dictionary and output DMAs ride the scalar
    HWDGE queue so the x stream never stalls.
  - The ones-column bias (x0/32) and the parity/t-tile interleave back
    to [B, T, D] are applied on the host.
"""

import numpy as np
from contextlib import ExitStack

import concourse.bass as bass
import concourse.bacc as bacc
import concourse.mybir as mybir
from concourse import tile
from concourse import bass_utils

F32 = mybir.dt.float32
F16 = mybir.dt.float16
F8E3 = mybir.dt.float8e3
AF = mybir.ActivationFunctionType
OP = mybir.AluOpType

N_CORES = 8
T = 1024
NP_ = 4096          # poles
B = 4
D = 2048
DSH = D // N_CORES  # 256 d columns per core
G_ = 16             # chunk pair groups (32 chunks of 128 poles)
NC_ = 32            # pole chunks
TT = 4              # t tiles of 128 per parity (512 t per parity)
BD = B * DSH        # 1024 (b,d) columns per core
DROP_BUDGET = (5e-4) ** 2 * 16385   # allowed dropped Frobenius mass
E3_CLIP = 14.0      # keep |x| under e3m4 max normal (15.5)


def _sched_layout(L):
    """Per-chunk kept t-rows (L[c]*32) -> tile widths and column offsets.

    Returns (tiles, cw): tiles[c] = list of (tt, width, off_a, off_b)
    and cw = total flat dictionary columns per parity.
    """
    tiles = []
    cw = 0
    for c in range(NC_):
        rows = 32 * L[c]
        tl = []
        tt = 0
        while rows > 0:
            w = min(128, rows)
            tl.append((tt, w, cw, cw + w))
            cw += 2 * w
            rows -= w
            tt += 1
        tiles.append(tl)
    return tiles, cw


def _quad_starts(tiles):
    """4-chunk supergroups made of single-tile 32-row chunks; these run
    column-tiled on the PE array (one chunk per 32-column strip)."""
    def small(c):
        return len(tiles[c]) == 1 and tiles[c][0][1] == 32

    return [c for c in range(4, NC_ - 3, 4)
            if all(small(c + k) for k in range(4))]


def build_kernel_nc(L):
    tiles, CW = _sched_layout(L)
    qstarts = _quad_starts(tiles)
    quadset = {qc + k for qc in qstarts for k in range(4)}
    first_c = {}
    last_c = {}
    for c in range(NC_):
        if c in quadset:
            continue
        for (tt, w, oa, ob_) in tiles[c]:
            first_c.setdefault(tt, c)
            last_c[tt] = c

    nc = bacc.Bacc("TRN2", target_bir_lowering=False, debug=False)

    ex_d = nc.dram_tensor("ex", [G_, 128, 2, 2, B, DSH], F8E3,
                          kind="ExternalInput")
    ox_d = nc.dram_tensor("ox", [G_, 128, 2, 2, B, DSH], F8E3,
                          kind="ExternalInput")
    dc_d = nc.dram_tensor("dc", [128, CW], F16, kind="ExternalInput")
    sc_d = nc.dram_tensor("sc", [128, 3, NC_], F32, kind="ExternalInput")
    # 0/1 matrix folding the 4 column-group partials back onto t-rows
    # 0:32: mg[p, r] = (p % 32 == r).
    mg_d = nc.dram_tensor("mg", [128, 32], F16, kind="ExternalInput")
    out_d = nc.dram_tensor("out", [2, 128, TT, BD], F16,
                           kind="ExternalOutput")

    with tile.TileContext(nc) as tc, ExitStack() as ctx:
        xp = ctx.enter_context(tc.tile_pool(name="xp", bufs=16))
        dp = ctx.enter_context(tc.tile_pool(name="dp", bufs=1))
        op_ = ctx.enter_context(tc.tile_pool(name="op", bufs=8))
        wp = ctx.enter_context(tc.tile_pool(name="wp", bufs=1))
        psp = ctx.enter_context(
            tc.tile_pool(name="ps", bufs=1, space=bass.MemorySpace.PSUM))

        # PE warm-up: HAM un-throttles (1.2 -> 2.4 GHz) only after ~3.4us
        # of sustained PE activity; burn the initial DMA window with
        # dummy matmuls (1-col stationary -> ~free LDWEIGHTS).
        wt = wp.tile([128, 257], F16, tag="wt", name="wt")
        nc.vector.memset(wt[:], 0.0)
        wps = psp.tile([128, 512], F32, tag="ps00", name="wps")
        for i in range(26):
            nc.tensor.matmul(wps[0:1, 0:256], wt[:, 0:1], wt[:, 1:257],
                             start=True, stop=True)

        # Dictionary loads ride the scalar queue so the sync queue streams
        # x back-to-back; the first chunks' tiles load first so the first
        # matmuls aren't gated on the full dictionary.
        nsplit = min(tiles[1][-1][3] if NC_ > 1 else CW, CW)

        dt0 = dp.tile([128, CW], F16, tag="d0", name="dt0")
        nc.scalar.dma_start(dt0[:, 0:nsplit], dc_d[:, 0:nsplit])
        if nsplit < CW:
            nc.scalar.dma_start(dt0[:, nsplit:], dc_d[:, nsplit:])
        sc_t = wp.tile([128, 3, NC_], F32, tag="sc", name="sc_t")
        nc.scalar.dma_start(sc_t[:], sc_d[:])
        mg_t = wp.tile([128, 32], F16, tag="mg", name="mg_t")
        nc.scalar.dma_start(mg_t[:], mg_d[:])

        # Derive the odd-parity dictionary on the (idle) vector engine via
        # angle addition -- saves 0.85 MB of HBM traffic:
        #   A_o = s1*A_e - s2*B_e,  B_o = s1*B_e + s3*A_e
        # with per-pole s1 = r cos(th), s2 = r sin(th) G3/G1,
        # s3 = r sin(th) G1/G3.
        dt1 = dp.tile([128, CW], F16, tag="d1", name="dt1")
        tp_ = ctx.enter_context(tc.tile_pool(name="tp", bufs=2))
        for c in range(NC_):
            for (tt, w, oa, ob_) in tiles[c]:
                tmp = tp_.tile([128, 128], F16, tag="tmp", name="tmp")
                nc.vector.tensor_scalar(tmp[:, 0:w], dt0[:, ob_:ob_ + w],
                                        sc_t[:, 1, c:c + 1], None,
                                        op0=OP.mult)
                nc.vector.scalar_tensor_tensor(
                    dt1[:, oa:oa + w], dt0[:, oa:oa + w],
                    sc_t[:, 0, c:c + 1], tmp[:, 0:w],
                    op0=OP.mult, op1=OP.subtract)
                tmp2 = tp_.tile([128, 128], F16, tag="tmp", name="tmp2")
                nc.vector.tensor_scalar(tmp2[:, 0:w], dt0[:, oa:oa + w],
                                        sc_t[:, 2, c:c + 1], None,
                                        op0=OP.mult)
                nc.vector.scalar_tensor_tensor(
                    dt1[:, ob_:ob_ + w], dt0[:, ob_:ob_ + w],
                    sc_t[:, 0, c:c + 1], tmp2[:, 0:w],
                    op0=OP.mult, op1=OP.add)
        dts = [dt0, dt1]

        for par, xd in enumerate((ex_d, ox_d)):
            # PSUM: per h, bank 0 holds t-rows 0:128 (tt0) and a 3-bank
            # tile holds tt1-3 contiguously so they evacuate in one op.
            # The tt1 bank doubles as the column-tiled partial bank for
            # the 32-row chunks once tt1-3 retire (first group), and the
            # tt3/h1 bank hosts tiny keep-warm matmuls.
            ps = [[None, None] for _ in range(TT)]
            tails = []
            for h in range(2):
                ps[0][h] = psp.tile([128, 512], F32, tag=f"ps0{h}",
                                    name=f"ps0{h}")
                tl3 = psp.tile([128, 3, 512], F32, tag=f"pst{h}",
                               name=f"pst{par}{h}")
                tails.append(tl3)
                for tt in range(1, TT):
                    ps[tt][h] = tl3[:, tt - 1]
            aux = [tails[h][:, 0] for h in range(2)]
            warm_ps = tails[1][:, 2]
            nquad = len(qstarts)
            tail_last = max([last_c.get(tt, 0) for tt in range(1, TT)],
                            default=0)
            dt = dts[par]
            if par == 1:
                # Keep HAM warm across the parity transition.
                for i in range(8):
                    nc.tensor.matmul(wps[0:1, 0:256], wt[:, 0:1],
                                     wt[:, 1:257], start=True, stop=True)
            for g in range(G_):
                xt = xp.tile([128, 2, 2, B, DSH], F8E3, tag="x", name="xt")
                last_g = par == 1 and g == G_ - 1
                if (par == 0 and g == 0) or last_g:
                    # Halve the first/last transfer so the adjacent
                    # matmuls wait on 0.25 MB, not 0.5 MB.
                    nc.sync.dma_start(xt[:, 0], xd[g, :, 0])
                    nc.sync.dma_start(xt[:, 1], xd[g, :, 1])
                else:
                    nc.sync.dma_start(xt[:], xd[g])
                c0g = 2 * g
                if c0g in qstarts or c0g - 2 in qstarts:
                    qc = c0g if c0g in qstarts else c0g - 2
                    qi = qstarts.index(qc)
                    if c0g in qstarts:
                        # first half of the quad: stash the tile, issue
                        # a keep-warm matmul while the second half lands.
                        # The moving operand reads the fresh x tile so
                        # the scheduler pins it into the DMA-wait gap.
                        pend = xt
                        if g >= 2:
                            nc.tensor.matmul(warm_ps[0:1, :],
                                             wt[:, 0:1], xt[:, 0, 0, 0:2],
                                             start=True, stop=True)
                        continue
                    xts = [pend, xt]

                    def wave(ab, h):
                        # one wave of 4 matmuls; each hits a distinct
                        # column group so the strips run concurrently.
                        for k in range(4):
                            c = qc + k
                            tt, w, oa, ob2 = tiles[c][0]
                            off = oa if ab == 0 else ob2
                            nc.tensor.matmul(
                                aux[h][32 * k:32 * k + w, :],
                                dt[:, off:off + w],
                                xts[k // 2][:, k % 2, ab,
                                            2 * h:2 * h + 2],
                                # first write of each column strip must
                                # clear its stale has_written bits (the
                                # clear is per-element, not per-bank).
                                start=(qi == 0 and ab == 0),
                                stop=(qi == nquad - 1 and ab == 1
                                      and k == 3),
                                tile_position=(0, 32 * k),
                            )

                    if qi < nquad - 1:
                        for ab in range(2):
                            for h in range(2):
                                wave(ab, h)
                        continue
                    # Last quad: run h-major and interleave the fold/
                    # evacuation chains so the vector CASTs overlap the
                    # other half's waves and the PE queue never waits on
                    # a CAST.  Dedicated tiles avoid pool-reuse waits.
                    axs = [wp.tile([128, 512], F16, tag=f"ax{h}",
                                   name=f"ax{h}") for h in range(2)]
                    for h in range(2):
                        wave(0, h)
                        wave(1, h)
                        nc.vector.tensor_copy(axs[h][:], aux[h][:, :])
                    for h in range(2):
                        nc.tensor.matmul(ps[0][h][0:32, :],
                                         mg_t[:, 0:32], axs[h][:],
                                         start=False, stop=True)
                    nq = 2 if par == 1 else 1
                    for h in range(2):
                        ob = wp.tile([128, 512], F16, tag=f"obt{h}",
                                     name=f"obt{h}")
                        for q in range(nq):
                            cs = slice(q * 512 // nq, (q + 1) * 512 // nq)
                            osl = out_d[par, :, 0, h * 512 + cs.start:
                                        h * 512 + cs.stop]
                            if h == 0:
                                nc.scalar.activation(
                                    ob[:, cs], ps[0][h][:, cs],
                                    AF.Identity, bias=0.0, scale=1.0)
                                nc.scalar.dma_start(osl, ob[:, cs])
                            else:
                                nc.vector.tensor_copy(ob[:, cs],
                                                      ps[0][h][:, cs])
                                eng = nc.sync if par == 1 else nc.scalar
                                eng.dma_start(osl, ob[:, cs])
                    continue
                for j in range(2):
                    c = c0g + j
                    tl = tiles[c]
                    if par == 1 and c == 0 and len(tl) > 1:
                        # tt0 last: its bank frees only after the prior
                        # parity's merge + evacuation, and the PE queue
                        # is in-order.
                        tl = tl[1:] + tl[:1]
                    for (tt, w, oa, ob2) in tl:
                        for ab, off in ((0, oa), (1, ob2)):
                            for h in range(2):
                                nc.tensor.matmul(
                                    ps[tt][h][0:w, :],
                                    dt[:, off:off + w],
                                    xt[:, j, ab, 2 * h:2 * h + 2],
                                    start=(c == first_c[tt] and ab == 0),
                                    stop=(c == last_c[tt] and ab == 1
                                          and not (tt == 0 and nquad)),
                                )
                    if c == tail_last and TT > 1:
                        # tt1-3 retire here (only the big chunks reach
                        # them): one 3-bank copy + one DMA per h, on the
                        # scalar engine -- the vector engine is deep in
                        # the odd-dictionary derivation at this point.
                        for h in range(2):
                            ob3 = op_.tile([128, 3, 512], F16,
                                           tag="ob3", name="ob3")
                            nc.scalar.activation(ob3[:], tails[h][:],
                                                 AF.Identity, bias=0.0,
                                                 scale=1.0)
                            nc.scalar.dma_start(
                                out_d[par, :, 1:TT,
                                      h * 512:(h + 1) * 512], ob3[:])
            if not nquad:
                # No column-tiled chunks (adversarial schedule): plain
                # tt0 evacuation.
                for h in range(2):
                    ob = op_.tile([128, 512], F16, tag="ob", name="ob")
                    osl = out_d[par, :, 0, h * 512:(h + 1) * 512]
                    if h == 0:
                        nc.scalar.activation(ob[:], ps[0][h][:],
                                             AF.Identity, bias=0.0,
                                             scale=1.0)
                        nc.scalar.dma_start(osl, ob[:])
                    else:
                        nc.vector.tensor_copy(ob[:], ps[0][h][:])
                        nc.scalar.dma_start(osl, ob[:])
    nc.compile()
    return nc


_NC_CACHE = {}


def _get_nc(L):
    key = tuple(L)
    if key not in _NC_CACHE:
        _NC_CACHE[key] = build_kernel_nc(L)
    return _NC_CACHE[key]


def _build_dict_halves(rr, theta):
    """Normalized Abar/Bbar [T, NP_] fp32, exactly as the reference."""
    i = np.arange(T, dtype=np.float32)[:, None]
    pw = rr[None, :] ** i
    ang = (i * theta[None, :]).astype(np.float32)
    c = np.cos(ang).astype(np.float32)
    s = np.sin(ang).astype(np.float32)
    sign = np.where(i % 2 == 0, np.float32(1.0), np.float32(-1.0))
    ones = np.ones((T, 1), np.float32)
    w1 = pw * c
    w3 = pw * s
    dic = np.concatenate([ones, w1, sign * w1, w3, sign * w3],
                         axis=1).astype(np.float32)
    G = np.linalg.norm(dic, axis=0)
    G = np.where(G == 0, np.sqrt(np.float32(T)), G).astype(np.float32)
    abar = dic[:, 1:1 + NP_] / G[None, 1:1 + NP_]
    bbar = dic[:, 1 + 2 * NP_:1 + 3 * NP_] / G[None, 1 + 2 * NP_:1 + 3 * NP_]
    return abar, bbar


def _schedule(abar, bbar):
    """Per-chunk kept prefix length (units of 32 per-parity t rows).

    Mass of (chunk, 32-row block) = sum over both parities of squared
    normalized entries (A and B blocks; the S*A / S*B blocks mirror
    them, scaling total and dropped mass alike).
    """
    sq = abar * abar + bbar * bbar                      # [T, NP_]
    m = sq.reshape(16, 64, NC_, 128).sum(axis=(1, 3))   # [block, chunk]
    tail = m[::-1].cumsum(axis=0)[::-1]                 # tail mass from block
    share = DROP_BUDGET / 2 / NC_
    L = []
    for c in range(NC_):
        keep = 16
        for l in range(1, 17):
            if l == 16 or tail[l, c] <= share:
                keep = l
                break
        L.append(keep)
    return L


def _pack_dict(abar, bbar, L):
    """-> [2par, 128p, CW] fp16 flat dictionary per the schedule."""
    tiles, CW = _sched_layout(L)
    at = np.ascontiguousarray(abar.T)   # [NP_ k, T]
    bt = np.ascontiguousarray(bbar.T)
    dc = np.empty((2, 128, CW), np.float16)
    for par in range(2):
        atp = at[:, par::2]             # [NP_, 512]
        btp = bt[:, par::2]
        for c in range(NC_):
            ks = slice(c * 128, (c + 1) * 128)
            for (tt, w, oa, ob_) in tiles[c]:
                ms = slice(tt * 128, tt * 128 + w)
                dc[par, :, oa:oa + w] = atp[ks, ms]
                dc[par, :, ob_:ob_ + w] = btp[ks, ms]
    return dc


def _pack_xside(a, b):
    """a,b [B, NP_, D] fp32 (sorted poles) -> per-core e3m4 parts
    [G_, 128p, 2j, 2uw, B, DSH]."""
    import ml_dtypes
    big = np.stack([a, b], axis=0)                     # [2uw, B, NP_, D]
    np.clip(big, -E3_CLIP, E3_CLIP, out=big)
    big = big.astype(ml_dtypes.float8_e3m4)
    r = big.reshape(2, B, G_, 2, 128, N_CORES, DSH)
    # (uw0, b1, g2, j3, p4, c5, d6) -> (c, g, p, j, uw, b, d)
    rt = np.ascontiguousarray(r.transpose(5, 2, 4, 3, 0, 1, 6))
    return [rt[c] for c in range(N_CORES)]


def kernel(rr, theta, x, trace=False, trace_kwargs=None):
    rr = np.ascontiguousarray(np.asarray(rr, dtype=np.float32))
    theta = np.ascontiguousarray(np.asarray(theta, dtype=np.float32))
    x = np.asarray(x, dtype=np.float32)

    order = np.argsort(-rr, kind="stable")
    abar, bbar = _build_dict_halves(rr, theta)
    abar = abar[:, order]
    bbar = bbar[:, order]
    L = _schedule(abar, bbar)
    dc = np.ascontiguousarray(_pack_dict(abar, bbar, L)[0])
    i = np.arange(T, dtype=np.float32)[:, None]
    pw = rr[None, :] ** i
    ang = (i * theta[None, :]).astype(np.float32)
    G1 = np.linalg.norm((pw * np.cos(ang)).astype(np.float32), axis=0)
    G3 = np.linalg.norm((pw * np.sin(ang)).astype(np.float32), axis=0)
    G1 = np.where(G1 == 0, np.float32(np.sqrt(T)), G1)
    G3 = np.where(G3 == 0, np.float32(np.sqrt(T)), G3)
    rs, ths = rr[order], theta[order]
    g1s, g3s = G1[order], G3[order]
    s1 = (rs * np.cos(ths)).astype(np.float32)
    sf = (rs * np.sin(ths)).astype(np.float32)
    sc = np.stack([s1, sf * g3s / g1s, sf * g1s / g3s], axis=0)
    sc = np.ascontiguousarray(
        sc.reshape(3, NC_, 128).transpose(2, 0, 1).astype(np.float32))

    x1 = x[:, 1:1 + NP_][:, order]
    x2 = x[:, 1 + NP_:1 + 2 * NP_][:, order]
    x3 = x[:, 1 + 2 * NP_:1 + 3 * NP_][:, order]
    x4 = x[:, 1 + 3 * NP_:1 + 4 * NP_][:, order]
    u, w = x1 + x2, x3 + x4
    v, z = x1 - x2, x3 - x4
    # e3m4 max normal is 15.5; iid gaussian absmax is ~8, but guard by
    # rescaling everything if an adversarial input would clip hard.
    amax = max(np.abs(a).max() for a in (u, w, v, z))
    s_glob = np.float32(1.0)
    if amax > E3_CLIP:
        s_glob = np.float32(amax / E3_CLIP)
        inv = np.float32(1.0 / s_glob)
        u = u * inv
        w = w * inv
        v = v * inv
        z = z * inv
    ex_cores = _pack_xside(u, w)
    ox_cores = _pack_xside(v, z)

    mg = np.zeros((128, 32), np.float16)
    mg[np.arange(128), np.arange(128) % 32] = np.float16(1.0)

    nc = _get_nc(L)
    in_maps = []
    for c in range(N_CORES):
        im = {"ex": ex_cores[c], "ox": ox_cores[c], "dc": dc, "sc": sc,
              "mg": mg}
        in_maps.append(im)
    kw = {}
    if trace:
        kw = {"trace": True, "trace_kwargs": trace_kwargs or {}}
    try:
        res = bass_utils.run_bass_kernel_spmd(
            nc, in_maps, core_ids=list(range(N_CORES)), **kw)
    except Exception:
        # Transient device wedge (e.g. NRT_EXEC_UNIT_UNRECOVERABLE) --
        # one retry usually clears it.
        res = bass_utils.run_bass_kernel_spmd(
            nc, in_maps, core_ids=list(range(N_CORES)), **kw)

    out = np.empty((B, T, D), dtype=np.float32)
    for c in range(N_CORES):
        oc = res.results[c]["out"]           # [2, 128, TT, BD]
        dsl = slice(c * DSH, (c + 1) * DSH)
        for par in range(2):
            for tt in range(TT):
                blk = oc[par, :, tt].reshape(128, B, DSH)
                blk = blk.transpose(1, 0, 2)
                out[:, 256 * tt + par:256 * (tt + 1):2, dsl] = blk
    if s_glob != 1.0:
        out *= s_glob
    out += x[:, 0:1, :] * np.float32(1.0 / 32.0)
    if trace:
        return out, res
    return out
